# revision 17
# baseline (speedup 1.0000x reference)
"""Bidirectional RoPE self-attention (Q is both query and key) on 8 trn2 cores.

Math (per (b,h) pair, T=1024, N=256):
    QR = rope(Q); S = QR @ QR.T / 16; out = softmax(S) @ V

Device strategy:
  - 96 (b,h) pairs sharded 12-per-core (batch/head parallel, no comm).
  - Host pre-transposes Q to [N, T] layout with even/odd channel
    deinterleave so RoPE needs no partition shuffles: channels [0::2] in
    the first 128 partitions, [1::2] in the second; rope is 6 aligned
    elementwise DVE ops using host-precomputed cos/sin tables (scaled by
    1/4 so scores come out pre-divided by sqrt(256)=16).
  - scores: fp32r matmuls (full PE rate at moving dim >= 256), contraction
    over the 2 channel tiles, PSUM tiles [128, 1024] (2 banks).
  - exp: one ScalarE activation per t-tile, PSUM -> SBUF, with accum_out
    producing the softmax row-sum Z for free (no max-subtraction needed:
    scores/16 <= ~22 so exp fits fp32 comfortably).
  - attn @ V, transposed: scores are symmetric, so stored E tiles [t, s]
    are also [s, t]; compute outT[n, t] = sum_s V[s, n] * E[s, t] with V
    slices as stationary weights (reused across two 512-wide moving E
    chunks, so the fp32r self-weight-load stays hidden) and E as the
    moving operand at full rate. Host un-transposes the output.
  - 1/Z: reciprocal of the accum column [128, 8], transpose-DMA it to a
    [1, T] row, gpsimd partition_broadcast to [128, T], multiply the outT
    PSUM tiles on DVE.
  - One merged DMA per pair for each of Q-load, V-load, out-store.
"""

from contextlib import ExitStack

import numpy as np

import concourse.bacc as bacc
import concourse.tile as tile
from concourse import mybir

B, NH, T, N = 8, 12, 1024, 256
NCORES = 8
PAIRS = B * NH // NCORES  # 12 (b,h) pairs per core
F32 = mybir.dt.float32
F32R = mybir.dt.float32r
EXP = mybir.ActivationFunctionType.Exp

NTT = T // 128  # 8 t-tiles (= s-chunks) per pair


def build_nc(pairs=PAIRS):
    nc = bacc.Bacc("TRN2", target_bir_lowering=False, debug=False,
                   enable_asserts=False)

    qt = nc.dram_tensor("qt", [pairs, 128, 2, T], F32, kind="ExternalInput")
    v = nc.dram_tensor("v", [pairs, 128, NTT, N], F32R, kind="ExternalInput")
    cs = nc.dram_tensor("cs", [2, 128, T], F32, kind="ExternalInput")
    onesd = nc.dram_tensor("ones", [1, 128], F32R, kind="ExternalInput")
    outt = nc.dram_tensor("outt", [pairs, 128, 2, T], F32, kind="ExternalOutput")

    with tile.TileContext(nc) as tc, ExitStack() as ctx:
        cpool = ctx.enter_context(tc.tile_pool(name="cs", bufs=1))
        qpool = ctx.enter_context(tc.tile_pool(name="q", bufs=2))
        tpool = ctx.enter_context(tc.tile_pool(name="tmp", bufs=2))
        qrpool = ctx.enter_context(tc.tile_pool(name="qr", bufs=2))
        epool = ctx.enter_context(tc.tile_pool(name="e", bufs=16))
        vpool = ctx.enter_context(tc.tile_pool(name="v", bufs=2))
        opool = ctx.enter_context(tc.tile_pool(name="o", bufs=2))
        zpool = ctx.enter_context(tc.tile_pool(name="z", bufs=2))
        ps_s = ctx.enter_context(tc.tile_pool(name="ps_s", bufs=2, space="PSUM"))
        ps_o = ctx.enter_context(tc.tile_pool(name="ps_o", bufs=2, space="PSUM"))

        ctile = cpool.tile([128, T], F32, tag="c")
        stile = cpool.tile([128, T], F32, tag="s")
        nc.sync.dma_start(ctile[:], cs[0])
        nc.sync.dma_start(stile[:], cs[1])
        ones1 = cpool.tile([1, 128], F32R, tag="ones1")
        nc.sync.dma_start(ones1[:], onesd[:])

        for p in range(pairs):
            # merged loads: q8 [128, 2T] (chunk-major), v8 [128, 8*N]
            q8 = qpool.tile([128, 2 * T], F32)
            nc.sync.dma_start(q8[:, 0:T], qt[p, :, 0, :])
            nc.scalar.dma_start(q8[:, T:2 * T], qt[p, :, 1, :])
            v8 = vpool.tile([128, NTT * N], F32R)
            nc.gpsimd.dma_start(v8[:].rearrange("p (c n) -> p c n", c=NTT), v[p])  # flat-order match
            q0, q1 = q8[:, 0:T], q8[:, T:2 * T]

            # rope: qr0 = q0*C - q1*S ; qr1 = q1*C + q0*S   (C,S carry 1/4)
            ta = tpool.tile([128, T], F32, tag="ta")
            tb = tpool.tile([128, T], F32, tag="tb")
            nc.vector.tensor_mul(ta[:], q0, ctile[:])
            nc.vector.tensor_mul(tb[:], q1, stile[:])
            qr8 = qrpool.tile([128, 2 * T], F32R)
            nc.vector.tensor_sub(qr8[:, 0:T], ta[:], tb[:])
            tc2 = tpool.tile([128, T], F32, tag="tc")
            td = tpool.tile([128, T], F32, tag="td")
            nc.gpsimd.tensor_mul(tc2[:], q1, ctile[:])
            nc.gpsimd.tensor_mul(td[:], q0, stile[:])
            nc.vector.tensor_add(qr8[:, T:2 * T], tc2[:], td[:])
            qrs = (qr8[:, 0:T], qr8[:, T:2 * T])

            # scores + exp (+row-sum Z) per t-tile
            zacc = zpool.tile([128, NTT], F32, tag="zacc")
            et = []
            for tt in range(NTT):
                ps = ps_s.tile([128, T], F32)
                for sc in range(T // 512):
                    for k in range(2):
                        nc.tensor.matmul(
                            ps[:, sc * 512:(sc + 1) * 512],
                            qrs[k][:, tt * 128:(tt + 1) * 128],
                            qrs[k][:, sc * 512:(sc + 1) * 512],
                            start=(k == 0), stop=(k == 1),
                        )
                e = epool.tile([128, T], F32R)
                nc.scalar.activation(e[:], ps[:], EXP,
                                     accum_out=zacc[:, tt:tt + 1])
                et.append(e)

            # 1/Z: flat-copy the [128, 8] accum to a [1, T] row (order is
            # j = p*8 + tt); later broadcast to [128, T] via a PE outer
            # product with ones + PSUM->SBUF copy.
            zrec = zpool.tile([128, NTT], F32R, tag="zrec")
            with nc.allow_low_precision(reason="fp32r 1/Z is plenty"):
                nc.vector.reciprocal(zrec[:], zacc[:])
            zrow = zpool.tile([1, T], F32R, tag="zrow")
            nc.sync.dma_start(
                zrow[0:1, :].rearrange("o (a b) -> o a b", a=128),
                zrec[:, :])
            zrb = zpool.tile([128, T], F32, tag="zrb")

            # outT[n, t] = sum_s V[s, n] E[s, t] / Z_t
            # (E[t,s] tiles reused as [s,t] via symmetry)
            o8 = opool.tile([128, 2 * T], F32)
            for nch in range(2):
                for tch in range(2):
                    po = ps_o.tile([128, 512], F32)
                    for c in range(NTT):
                        nc.tensor.matmul(
                            po[:],
                            v8[:, c * N + nch * 128: c * N + nch * 128 + 128],
                            et[c][:, tch * 512:(tch + 1) * 512],
                            start=(c == 0), stop=(c == NTT - 1),
                        )
                    if nch == 0 and tch == 0:
                        # zrow is long ready here; PE hits these without
                        # stalling and DVE gets zrb before the first scale
                        for j in range(2):
                            pz = ps_o.tile([128, 512], F32, tag="pz")
                            nc.tensor.matmul(pz[:], ones1[0:1, :],
                                             zrow[0:1, j * 512:(j + 1) * 512],
                                             start=True, stop=True)
                            nc.scalar.copy(
                                zrb[:, j * 512:(j + 1) * 512], pz[:])
                    off = nch * T + tch * 512
                    # zrb free layout is j = p*8 + tt; po column u*128 + p
                    # needs Z[tt = 4*tch + u, p] -> strided view
                    zv = zrb[:].rearrange("q (p t) -> q t p", p=128)
                    nc.vector.tensor_mul(o8[:, off:off + 512], po[:],
                                         zv[:, 4 * tch:4 * tch + 4, :])
                if nch == 0:
                    nc.sync.dma_start(
                        outt[p, :, 0:1, :],
                        o8[:, 0:T].rearrange("p (k t) -> p k t", k=1))
            nc.scalar.dma_start(
                outt[p, :, 1:2, :],
                o8[:, T:2 * T].rearrange("p (k t) -> p k t", k=1))

    nc.compile()
    return nc


def host_prep(Q, V, freqs):
    """Returns per-core in_maps for the 8 cores."""
    Q = np.ascontiguousarray(np.asarray(Q), dtype=np.float32)
    V = np.ascontiguousarray(np.asarray(V), dtype=np.float32)
    freqs = np.asarray(freqs, dtype=np.float32)

    # cos/sin tables in [channel-pair, t] layout, scaled by 1/4.
    half = freqs.reshape(-1)[0::2]  # [128] cycles-per-step
    t_col = np.arange(T, dtype=np.float32).reshape(T, 1)
    phases = t_col * half.reshape(1, 128)  # [T, 128] fp32
    ang = np.mod(phases, np.float32(1.0)) * np.float32(2.0 * np.pi)
    C = (np.cos(ang).astype(np.float32) * np.float32(0.25)).T  # [128, T]
    S = (np.sin(ang).astype(np.float32) * np.float32(0.25)).T
    cs_np = np.ascontiguousarray(np.stack([C, S]))  # [2, 128, T]

    G = B * NH
    Qg = Q.reshape(G, T, N)
    QT = np.empty((G, 128, 2, T), np.float32)
    QT[:, :, 0] = Qg[:, :, 0::2].transpose(0, 2, 1)  # even channels
    QT[:, :, 1] = Qg[:, :, 1::2].transpose(0, 2, 1)  # odd channels
    # v dram [g, s%128 (partition), s//128 (chunk), n]
    Vg = np.ascontiguousarray(
        V.reshape(G, NTT, 128, N).transpose(0, 2, 1, 3))

    in_maps = []
    for c in range(NCORES):
        sl = slice(c * PAIRS, (c + 1) * PAIRS)
        in_maps.append({"qt": QT[sl], "v": Vg[sl], "cs": cs_np,
                        "ones": np.ones((1, 128), np.float32)})
    return in_maps


_CACHED_NC = None


def kernel(Q, V, freqs):
    global _CACHED_NC
    from concourse.bass_utils import run_bass_kernel_spmd

    in_maps = host_prep(Q, V, freqs)
    if _CACHED_NC is None:
        _CACHED_NC = build_nc()
    res = run_bass_kernel_spmd(_CACHED_NC, in_maps, list(range(NCORES)))
    # outt [pairs, 128 (n%128), 2 (n//128), T] -> [g, T, N]
    outs = [res.results[c]["outt"] for c in range(NCORES)]
    full = np.concatenate(outs)  # [96, 128, 2, T]
    full = full.transpose(0, 3, 2, 1).reshape(B * NH, T, N)  # n = k*128 + p
    return np.ascontiguousarray(full).reshape(B, NH, T, N)


# revision 18
# speedup vs baseline: 1.2202x; 1.2202x over previous
"""Bidirectional RoPE self-attention (Q is both query and key) on 8 trn2 cores.

Math (per (b,h) pair, T=1024, N=256):
    QR = rope(Q); S = QR @ QR.T / 16; out = softmax(S) @ V

Device strategy:
  - 96 (b,h) pairs sharded 12-per-core (batch/head parallel, no comm).
  - Host pre-transposes Q to [N, T] bf16 with even/odd channel
    deinterleave so RoPE needs no partition shuffles; rope is 6 aligned
    elementwise DVE ops (bf16, 2x rate) using host-precomputed bf16
    cos/sin tables scaled by 1/4 (folds the 1/sqrt(256) softmax scale),
    writing QR as fp8e4m3.
  - scores: one fp8 DoubleRow matmul per (t-tile, s-chunk): K=256 in a
    single pass via the [Ki, 2, *] interleave over the two channel
    chunks. Scores land in fp32 PSUM [128, 1024] (2 banks).
  - exp: one ScalarE activation per t-tile, PSUM -> SBUF fp32r E tiles,
    with accum_out producing the softmax row-sum Z for free (no
    max-subtraction: scores/16 <= ~22 fits fp32 exp comfortably).
  - attn @ V, transposed: scores are symmetric, so stored E tiles [t, s]
    are also [s, t]; outT[n, t] = sum_s V[s, n] E[s, t] with V slices as
    fp32r stationary weights and E as the fp32r moving operand (full PE
    rate at moving dim 512). Host un-transposes the output.
  - 1/Z: reciprocal of the accum column [128, 8], flat-DMA to a [1, T]
    row (order j = p*8 + tt), PE outer-product broadcast with ones to
    [128, T]; the final DVE scale reads it through a matching strided
    view.
  - DMA rings: q8/cs/zrow/out-half0 on sync, v8 on gpsimd, out-half1 on
    scalar; one merged DMA per pair per tensor.
"""

from contextlib import ExitStack

import numpy as np

import concourse.bacc as bacc
import concourse.tile as tile
from concourse import mybir

B, NH, T, N = 8, 12, 1024, 256
NCORES = 8
PAIRS = B * NH // NCORES  # 12 (b,h) pairs per core
F32 = mybir.dt.float32
F32R = mybir.dt.float32r
BF16 = mybir.dt.bfloat16
FP8 = mybir.dt.float8e4
EXP = mybir.ActivationFunctionType.Exp
DR = mybir.MatmulPerfMode.DoubleRow

NTT = T // 128  # 8 t-tiles (= s-chunks) per pair


def build_nc(pairs=PAIRS):
    nc = bacc.Bacc("TRN2", target_bir_lowering=False, debug=False,
                   enable_asserts=False)

    qt = nc.dram_tensor("qt", [pairs, 128, 2, T], BF16, kind="ExternalInput")
    v = nc.dram_tensor("v", [pairs, 128, NTT, N], F32R, kind="ExternalInput")
    cs = nc.dram_tensor("cs", [2, 128, T], BF16, kind="ExternalInput")
    onesd = nc.dram_tensor("ones", [1, 128], F32R, kind="ExternalInput")
    outt = nc.dram_tensor("outt", [pairs, 128, 2, T], F32, kind="ExternalOutput")

    with tile.TileContext(nc) as tc, ExitStack() as ctx:
        cpool = ctx.enter_context(tc.tile_pool(name="cs", bufs=1))
        qpool = ctx.enter_context(tc.tile_pool(name="q", bufs=2))
        tpool = ctx.enter_context(tc.tile_pool(name="tmp", bufs=2))
        qrpool = ctx.enter_context(tc.tile_pool(name="qr", bufs=2))
        epool = ctx.enter_context(tc.tile_pool(name="e", bufs=16))
        vpool = ctx.enter_context(tc.tile_pool(name="v", bufs=2))
        opool = ctx.enter_context(tc.tile_pool(name="o", bufs=2))
        zpool = ctx.enter_context(tc.tile_pool(name="z", bufs=2))
        ps_s = ctx.enter_context(tc.tile_pool(name="ps_s", bufs=2, space="PSUM"))
        ps_o = ctx.enter_context(tc.tile_pool(name="ps_o", bufs=2, space="PSUM"))

        ctile = cpool.tile([128, T], BF16, tag="c")
        stile = cpool.tile([128, T], BF16, tag="s")
        nc.sync.dma_start(ctile[:], cs[0])
        nc.sync.dma_start(stile[:], cs[1])
        ones1 = cpool.tile([1, 128], F32R, tag="ones1")
        nc.sync.dma_start(ones1[:], onesd[:])

        for p in range(pairs):
            # merged loads: q8 [128, 2T] bf16 (k-chunk major), v8 [128, 8*N]
            q8 = qpool.tile([128, 2 * T], BF16)
            nc.sync.dma_start(q8[:].rearrange("p (k t) -> p k t", k=2), qt[p])
            v8 = vpool.tile([128, NTT * N], F32R)
            nc.gpsimd.dma_start(v8[:].rearrange("p (c n) -> p c n", c=NTT), v[p])
            q0, q1 = q8[:, 0:T], q8[:, T:2 * T]

            # rope: qr0 = q0*C - q1*S ; qr1 = q1*C + q0*S   (C,S carry 1/4)
            ta = tpool.tile([128, T], BF16, tag="ta")
            tb = tpool.tile([128, T], BF16, tag="tb")
            nc.vector.tensor_mul(ta[:], q0, ctile[:])
            nc.vector.tensor_mul(tb[:], q1, stile[:])
            qr8 = qrpool.tile([128, 2 * T], FP8)
            nc.vector.tensor_sub(qr8[:, 0:T], ta[:], tb[:])
            tc2 = tpool.tile([128, T], BF16, tag="ta")
            td = tpool.tile([128, T], BF16, tag="tb")
            nc.vector.tensor_mul(tc2[:], q1, ctile[:])
            nc.vector.tensor_mul(td[:], q0, stile[:])
            nc.vector.tensor_add(qr8[:, T:2 * T], tc2[:], td[:])
            # [ki, j, t] view for the DoubleRow K=256 contraction
            qr3 = qr8[:].rearrange("p (j t) -> p j t", j=2)

            # scores + exp (+row-sum Z) per t-tile
            zacc = zpool.tile([128, NTT], F32, tag="zacc")
            et = []
            for tt in range(NTT):
                ps = ps_s.tile([128, T], F32)
                for sc in range(T // 512):
                    nc.tensor.matmul(
                        ps[:, sc * 512:(sc + 1) * 512],
                        qr3[:, :, tt * 128:(tt + 1) * 128],
                        qr3[:, :, sc * 512:(sc + 1) * 512],
                        start=True, stop=True, perf_mode=DR,
                    )
                e = epool.tile([128, T], F32R)
                nc.scalar.activation(e[:], ps[:], EXP,
                                     accum_out=zacc[:, tt:tt + 1])
                et.append(e)

            # 1/Z: flat-copy the [128, 8] accum to a [1, T] row (order is
            # j = p*8 + tt); later broadcast to [128, T] via a PE outer
            # product with ones + PSUM->SBUF copy.
            zrec = zpool.tile([128, NTT], F32R, tag="zrec")
            with nc.allow_low_precision(reason="fp32r 1/Z is plenty"):
                nc.vector.reciprocal(zrec[:], zacc[:])
            zrow = zpool.tile([1, T], F32R, tag="zrow")
            nc.sync.dma_start(
                zrow[0:1, :].rearrange("o (a b) -> o a b", a=128),
                zrec[:, :])
            zrb = zpool.tile([128, T], F32, tag="zrb")

            # outT[n, t] = sum_s V[s, n] E[s, t] / Z_t
            # (E[t,s] tiles reused as [s,t] via symmetry)
            o8 = opool.tile([128, 2 * T], F32)
            for nch in range(2):
                for tch in range(2):
                    po = ps_o.tile([128, 512], F32)
                    for c in range(NTT):
                        nc.tensor.matmul(
                            po[:],
                            v8[:, c * N + nch * 128: c * N + nch * 128 + 128],
                            et[c][:, tch * 512:(tch + 1) * 512],
                            start=(c == 0), stop=(c == NTT - 1),
                        )
                    if nch == 0 and tch == 0:
                        # zrow is long ready here; PE hits these without
                        # stalling and DVE gets zrb before the first scale
                        for j in range(2):
                            pz = ps_o.tile([128, 512], F32, tag="pz")
                            nc.tensor.matmul(pz[:], ones1[0:1, :],
                                             zrow[0:1, j * 512:(j + 1) * 512],
                                             start=True, stop=True)
                            nc.vector.tensor_copy(
                                zrb[:, j * 512:(j + 1) * 512], pz[:])
                    off = nch * T + tch * 512
                    # zrb free layout is j = p*8 + tt; po column u*128 + p
                    # needs Z[tt = 4*tch + u, p] -> strided view
                    zv = zrb[:].rearrange("q (p t) -> q t p", p=128)
                    nc.vector.tensor_mul(o8[:, off:off + 512], po[:],
                                         zv[:, 4 * tch:4 * tch + 4, :])
                if nch == 0:
                    nc.sync.dma_start(
                        outt[p, :, 0:1, :],
                        o8[:, 0:T].rearrange("p (k t) -> p k t", k=1))
            nc.scalar.dma_start(
                outt[p, :, 1:2, :],
                o8[:, T:2 * T].rearrange("p (k t) -> p k t", k=1))

    nc.compile()
    return nc


def host_prep(Q, V, freqs):
    """Returns per-core in_maps for the 8 cores."""
    import ml_dtypes
    bf16 = ml_dtypes.bfloat16

    Q = np.ascontiguousarray(np.asarray(Q), dtype=np.float32)
    V = np.ascontiguousarray(np.asarray(V), dtype=np.float32)
    freqs = np.asarray(freqs, dtype=np.float32)

    # cos/sin tables in [channel-pair, t] layout, scaled by 1/4.
    half = freqs.reshape(-1)[0::2]  # [128] cycles-per-step
    t_col = np.arange(T, dtype=np.float32).reshape(T, 1)
    phases = t_col * half.reshape(1, 128)  # [T, 128] fp32
    ang = np.mod(phases, np.float32(1.0)) * np.float32(2.0 * np.pi)
    C = (np.cos(ang).astype(np.float32) * np.float32(0.25)).T  # [128, T]
    S = (np.sin(ang).astype(np.float32) * np.float32(0.25)).T
    cs_np = np.ascontiguousarray(np.stack([C, S])).astype(bf16)

    G = B * NH
    Qg = Q.reshape(G, T, N)
    QT = np.empty((G, 128, 2, T), bf16)
    QT[:, :, 0] = Qg[:, :, 0::2].transpose(0, 2, 1)  # even channels
    QT[:, :, 1] = Qg[:, :, 1::2].transpose(0, 2, 1)  # odd channels
    # v dram [g, s%128 (partition), s//128 (chunk), n]
    Vg = np.ascontiguousarray(
        V.reshape(G, NTT, 128, N).transpose(0, 2, 1, 3))

    in_maps = []
    for c in range(NCORES):
        sl = slice(c * PAIRS, (c + 1) * PAIRS)
        in_maps.append({"qt": QT[sl], "v": Vg[sl], "cs": cs_np,
                        "ones": np.ones((1, 128), np.float32)})
    return in_maps


_CACHED_NC = None


def kernel(Q, V, freqs):
    global _CACHED_NC
    from concourse.bass_utils import run_bass_kernel_spmd

    in_maps = host_prep(Q, V, freqs)
    if _CACHED_NC is None:
        _CACHED_NC = build_nc()
    res = run_bass_kernel_spmd(_CACHED_NC, in_maps, list(range(NCORES)))
    # outt [pairs, 128 (n%128), 2 (n//128), T] -> [g, T, N]
    outs = [res.results[c]["outt"] for c in range(NCORES)]
    full = np.concatenate(outs)  # [96, 128, 2, T]
    full = full.transpose(0, 3, 2, 1).reshape(B * NH, T, N)  # n = k*128 + p
    return np.ascontiguousarray(full).reshape(B, NH, T, N)
